# revision 8
# baseline (speedup 1.0000x reference)
"""Differential-Transformer attention (DiffAttn) Trainium2 Bass kernel.

Sharding: 8 cores = 2 (batch) x 4 (head-group tensor parallel).
Core c = 4*b + t handles batch b, query heads 4t..4t+3, kv head t,
and the two v-heads its query heads need (t//2 and t//2+2).
o_proj is row-parallel: each core returns a partial [L, HID] product;
the host sums the 4 partials per batch (the "unshard" step).

All matmuls run as float32r (fp32 data, reduced-precision multiply,
full PE speed at moving-dim >= 256). Softmax is computed without
max-subtraction (score magnitudes are bounded ~5, exp is safe in fp32)
on transposed score tiles S^T[k, q] so that the AV matmul needs no
transposes. The softmax denominator and the RMS-norm sum-of-squares
are computed with all-ones stationary matmuls which replicate the
result across all 128 partitions, so the combined normalization scale
  comb = rsqrt(ssq + 128*eps*den^2)       (algebraically exact fold of
                                           1/den, RMS rsqrt, and eps)
is computed on full [128, 512] tiles; rsqrt is exp(-0.5*ln(r)) so the
scalar engine only ever uses one LUT table set (exp/ln/copy/square).
"""

import os
import sys

import numpy as np

for _p in ("/opt/trn_rl_repo",):
    if _p not in sys.path and os.path.isdir(_p):
        sys.path.insert(0, _p)

B = 2
L = 2048
HID = 2048
D = 128
H = 16
NH = 4            # query heads per core
CT = HID // 128   # contraction tiles for the projections
EPS = 1e-6
LAMBDA_INIT = 0.2

_CACHE = {}


def _build(length=L):
    from concourse import bacc
    import concourse.mybir as mybir
    import concourse.tile as tile

    f32 = mybir.dt.float32
    f32r = mybir.dt.float32r
    Act = mybir.ActivationFunctionType

    NJ = length // 512    # q-slices
    NLB = length // 128   # l/k blocks

    nc = bacc.Bacc()
    hsT = nc.dram_tensor("hsT", [HID, length], f32, kind="ExternalInput")
    cosT = nc.dram_tensor("cosT", [D, length], f32, kind="ExternalInput")
    sinT = nc.dram_tensor("sinT", [D, length], f32, kind="ExternalInput")
    wq = nc.dram_tensor("wq", [HID, NH * D], f32, kind="ExternalInput")
    wk = nc.dram_tensor("wk", [HID, D], f32, kind="ExternalInput")
    wv = nc.dram_tensor("wv", [HID, 2 * D], f32, kind="ExternalInput")
    wo = nc.dram_tensor("wo", [NH * D, HID], f32, kind="ExternalInput")
    lam = nc.dram_tensor("lam", [D, 1], f32, kind="ExternalInput")
    rmsw = nc.dram_tensor("rmsw", [D, 1], f32, kind="ExternalInput")
    masks = nc.dram_tensor("masks", [4, D, 512], f32, kind="ExternalInput")
    part = nc.dram_tensor("part", [length, HID], f32, kind="ExternalOutput")

    inv_sqrt_d = 1.0 / np.sqrt(np.float32(D))

    with tile.TileContext(nc) as tc:
        with tc.tile_pool(name="persist", bufs=1) as persist:
            qT = persist.tile([D, NH, length], f32r, tag="qT")
            kT = persist.tile([D, length], f32r, tag="kT")
            veff = persist.tile([D, NLB, D], f32r, tag="veff")
            mask_t = persist.tile([D, 4, 512], f32r, tag="mask")
            lam_t = persist.tile([D, 1], f32, tag="lam")
            rmsw_t = persist.tile([D, 1], f32, tag="rmsw")
            ones_t = persist.tile([D, D], f32r, tag="ones")
            ones_stage = persist.tile([D, D], f32, tag="ones_stage")

            nc.sync.dma_start(out=mask_t.bitcast(f32),
                              in_=masks.rearrange("m p q -> p m q"))
            nc.sync.dma_start(out=lam_t, in_=lam[:, :])
            nc.sync.dma_start(out=rmsw_t, in_=rmsw[:, :])
            nc.vector.memset(ones_stage, 1.0)
            nc.vector.tensor_copy(out=ones_t, in_=ones_stage)

            # ---------------- Phase B: projections + RoPE -----------------
            with tc.tile_pool(name="wpool", bufs=1) as wpool, \
                 tc.tile_pool(name="hspool", bufs=20) as hspool, \
                 tc.tile_pool(name="stpool", bufs=6) as stpool, \
                 tc.tile_pool(name="cspool", bufs=2) as cspool, \
                 tc.tile_pool(name="btmp", bufs=4) as btmp, \
                 tc.tile_pool(name="bpsum", bufs=3, space="PSUM") as bpsum, \
                 tc.tile_pool(name="vpsum", bufs=2, space="PSUM") as vpsum:

                wq_t = wpool.tile([D, CT, NH * D], f32r, tag="wq")
                wk_t = wpool.tile([D, CT, D], f32r, tag="wk")
                wv_t = wpool.tile([D, CT, 2 * D], f32r, tag="wv")
                wq_r = wq.rearrange("(c p) m -> p c m", p=D)
                wk_r = wk.rearrange("(c p) m -> p c m", p=D)
                wv_r = wv.rearrange("(c p) m -> p c m", p=D)
                for c in range(CT):
                    st = stpool.tile([D, 512], f32, tag="stage")
                    nc.sync.dma_start(out=st, in_=wq_r[:, c, :])
                    nc.vector.tensor_copy(out=wq_t[:, c, :], in_=st)
                    st = stpool.tile([D, 512], f32, tag="stage")
                    nc.sync.dma_start(out=st[:, 0:D], in_=wk_r[:, c, :])
                    nc.sync.dma_start(out=st[:, D:3 * D], in_=wv_r[:, c, :])
                    nc.vector.tensor_copy(out=wk_t[:, c, :], in_=st[:, 0:D])
                    nc.vector.tensor_copy(out=wv_t[:, c, :], in_=st[:, D:3 * D])

                for j in range(NJ):
                    sl = slice(512 * j, 512 * (j + 1))
                    cos_s = cspool.tile([D, 512], f32, tag="cos")
                    sin_s = cspool.tile([D, 512], f32, tag="sin")
                    nc.sync.dma_start(out=cos_s, in_=cosT[:, sl])
                    nc.sync.dma_start(out=sin_s, in_=sinT[:, sl])

                    chunks = []
                    for c in range(CT):
                        st = stpool.tile([D, 512], f32, tag="stage")
                        ch = hspool.tile([D, 512], f32r, tag="hs")
                        nc.sync.dma_start(
                            out=st, in_=hsT[128 * c:128 * (c + 1), sl])
                        nc.vector.tensor_copy(out=ch, in_=st)
                        chunks.append(ch)

                    # q heads + k head in [D, l] (transposed) layout
                    for db in range(NH + 1):
                        ps = bpsum.tile([D, 512], f32, tag="qk")
                        for c in range(CT):
                            lhsT = (wq_t[:, c, 128 * db:128 * (db + 1)]
                                    if db < NH else wk_t[:, c, :])
                            nc.tensor.matmul(ps, lhsT, chunks[c],
                                             start=(c == 0), stop=(c == CT - 1))
                        # RoPE: rotate_half swaps partition halves; sin's two
                        # halves are identical (emb = concat([freqs, freqs])),
                        # so rot(q)*sin lands via partition-swapped reads of
                        # the PSUM tile (PSUM+SBUF input pairs are exempt from
                        # the same-base-partition rule).
                        t1 = btmp.tile([D, 512], f32, tag="t1")
                        t2 = btmp.tile([D, 512], f32, tag="t2")
                        nc.vector.tensor_mul(t1, ps, cos_s)
                        nc.vector.tensor_mul(t2[0:64, :], ps[64:128, :],
                                             sin_s[0:64, :])
                        nc.vector.tensor_mul(t2[64:128, :], ps[0:64, :],
                                             sin_s[64:128, :])
                        if db < NH:
                            dst_lo = qT[0:64, db, sl]
                            dst_hi = qT[64:128, db, sl]
                        else:
                            dst_lo = kT[0:64, sl]
                            dst_hi = kT[64:128, sl]
                        nc.vector.tensor_sub(dst_lo, t1[0:64, :], t2[0:64, :])
                        nc.vector.tensor_add(dst_hi, t1[64:128, :],
                                             t2[64:128, :])

                    # v in natural [l, d] layout; combine v_lo - lam*v_hi
                    for i in range(4):
                        kb = 4 * j + i
                        psv = vpsum.tile([D, 2 * D], f32, tag="v")
                        for c in range(CT):
                            nc.tensor.matmul(
                                psv, chunks[c][:, 128 * i:128 * (i + 1)],
                                wv_t[:, c, :],
                                start=(c == 0), stop=(c == CT - 1))
                        tv = btmp.tile([D, D], f32, tag="tv")
                        nc.vector.tensor_scalar_mul(tv, psv[:, D:2 * D], lam_t)
                        nc.vector.tensor_sub(veff[:, kb, :], psv[:, 0:D], tv)

            # ---------------- Phase C + D ---------------------------------
            with tc.tile_pool(name="cdpool", bufs=1) as cdpool:
                finalT = cdpool.tile([D, NH, length], f32r, tag="finalT")

                with tc.tile_pool(name="cpool", bufs=6) as cpool, \
                     tc.tile_pool(name="ctmp", bufs=3) as ctmp, \
                     tc.tile_pool(name="pso_p", bufs=2, space="PSUM") as pso_p, \
                     tc.tile_pool(name="pss_p", bufs=3, space="PSUM") as pss_p, \
                     tc.tile_pool(name="psd_p", bufs=2, space="PSUM") as psd_p, \
                     tc.tile_pool(name="psq_p", bufs=1, space="PSUM") as psq_p:
                    for j in range(NJ):
                        sl = slice(512 * j, 512 * (j + 1))
                        nkb = 4 * j + 4
                        for h in range(NH):
                            pso = pso_p.tile([D, 512], f32, tag="o")
                            psden = psd_p.tile([D, 512], f32, tag="den")
                            # software-pipelined: emit score(kb+1) before
                            # den/AV(kb) so PE never waits on ACT's exp
                            se_l = {}

                            def score(kb):
                                pss = pss_p.tile([D, 512], f32, tag="s")
                                nc.tensor.matmul(
                                    pss, kT[:, 128 * kb:128 * (kb + 1)],
                                    qT[:, h, sl], start=True, stop=True)
                                se = cpool.tile([D, 512], f32r, tag="se")
                                nc.scalar.activation(se, pss, Act.Exp,
                                                     scale=float(inv_sqrt_d))
                                if kb >= 4 * j:
                                    nc.vector.tensor_mul(
                                        se, se, mask_t[:, kb - 4 * j, :])
                                se_l[kb] = se

                            score(0)
                            for kb in range(nkb):
                                if kb + 1 < nkb:
                                    score(kb + 1)
                                se = se_l.pop(kb)
                                nc.tensor.matmul(psden, ones_t, se,
                                                 start=(kb == 0),
                                                 stop=(kb == nkb - 1))
                                nc.tensor.matmul(pso, veff[:, kb, :], se,
                                                 start=(kb == 0),
                                                 stop=(kb == nkb - 1))

                            # comb = rsqrt(ssq + 128*eps*den^2), all [128,512]
                            sq = cpool.tile([D, 512], f32r, tag="sq")
                            nc.scalar.activation(sq, pso, Act.Square)
                            psss = psq_p.tile([D, 512], f32, tag="ss")
                            nc.tensor.matmul(psss, ones_t, sq,
                                             start=True, stop=True)
                            c1 = ctmp.tile([D, 512], f32, tag="c1")
                            nc.scalar.activation(c1, psden, Act.Square)
                            nc.vector.tensor_scalar_mul(c1, c1, 128.0 * EPS)
                            nc.vector.tensor_add(c1, c1, psss)
                            lnr = ctmp.tile([D, 512], f32, tag="lnr")
                            nc.scalar.activation(lnr, c1, Act.Ln)
                            comb = ctmp.tile([D, 512], f32, tag="comb")
                            nc.scalar.activation(comb, lnr, Act.Exp, scale=-0.5)
                            ft = ctmp.tile([D, 512], f32, tag="ft")
                            nc.vector.tensor_mul(ft, pso, comb)
                            nc.vector.tensor_scalar_mul(
                                finalT[:, h, sl], ft, rmsw_t)

                # ---------------- Phase D: o_proj ------------------------
                with tc.tile_pool(name="dpool", bufs=1) as dpool, \
                     tc.tile_pool(name="dout", bufs=4) as dout, \
                     tc.tile_pool(name="dpsum", bufs=2, space="PSUM") as dpsum:
                    wo_t = dpool.tile([D, NH, HID], f32r, tag="wo")
                    wo_r = wo.rearrange("(h p) m -> p h m", p=D)
                    for h in range(NH):
                        for q4 in range(HID // 512):
                            st = dout.tile([D, 512], f32, tag="wost")
                            nc.sync.dma_start(
                                out=st, in_=wo_r[:, h, 512 * q4:512 * (q4 + 1)])
                            nc.vector.tensor_copy(
                                out=wo_t[:, h, 512 * q4:512 * (q4 + 1)], in_=st)
                    for lb in range(NLB):
                        lsl = slice(128 * lb, 128 * (lb + 1))
                        for osl in range(HID // 512):
                            osl_s = slice(512 * osl, 512 * (osl + 1))
                            ps = dpsum.tile([D, 512], f32, tag="op")
                            for h in range(NH):
                                nc.tensor.matmul(ps, finalT[:, h, lsl],
                                                 wo_t[:, h, osl_s],
                                                 start=(h == 0), stop=(h == NH - 1))
                            ob = dout.tile([D, 512], f32, tag="ob")
                            nc.scalar.copy(ob, ps)
                            nc.sync.dma_start(out=part[lsl, osl_s], in_=ob)

    nc.finalize()
    return nc


def _causal_masks():
    m = np.zeros((4, D, 512), np.float32)
    for r in range(4):
        for p in range(D):
            q0 = 128 * r + p
            if q0 < 512:
                m[r, p, q0:] = 1.0
    return m


def kernel(hidden_states, cos, sin, Wq, Wk, Wv, Wo,
           lambda_q1, lambda_k1, lambda_q2, lambda_k2, rms_weight):
    from concourse.bass_utils import run_bass_kernel_spmd

    length = hidden_states.shape[1]
    if length not in _CACHE:
        _CACHE[length] = _build(length)
    nc = _CACHE[length]

    hidden_states = np.asarray(hidden_states, np.float32)
    cos = np.asarray(cos, np.float32)
    sin = np.asarray(sin, np.float32)

    lam_full = np.float32(
        np.exp(np.float32(np.dot(np.asarray(lambda_q1, np.float32),
                                 np.asarray(lambda_k1, np.float32)))
               + np.float32(np.dot(np.asarray(lambda_q2, np.float32),
                                   np.asarray(lambda_k2, np.float32))))
        + np.float32(LAMBDA_INIT))
    lam_arr = np.full((D, 1), lam_full, np.float32)
    rmsw_arr = np.ascontiguousarray(
        (np.asarray(rms_weight, np.float32)
         * np.float32((1.0 - LAMBDA_INIT) * np.sqrt(128.0))).reshape(D, 1))
    masks = _causal_masks()

    Wq = np.asarray(Wq, np.float32)
    Wk = np.asarray(Wk, np.float32)
    Wv = np.asarray(Wv, np.float32)
    Wo = np.asarray(Wo, np.float32)

    in_maps = []
    for b in range(B):
        hsT_b = np.ascontiguousarray(hidden_states[b].T)
        cosT_b = np.ascontiguousarray(cos[b].T)
        sinT_b = np.ascontiguousarray(sin[b].T)
        for t in range(4):
            vlo, vhi = t // 2, t // 2 + 2
            in_maps.append({
                "hsT": hsT_b,
                "cosT": cosT_b,
                "sinT": sinT_b,
                "wq": np.ascontiguousarray(Wq[:, 512 * t:512 * (t + 1)]),
                "wk": np.ascontiguousarray(Wk[:, 128 * t:128 * (t + 1)]),
                "wv": np.ascontiguousarray(np.concatenate(
                    [Wv[:, 128 * vlo:128 * (vlo + 1)],
                     Wv[:, 128 * vhi:128 * (vhi + 1)]], axis=1)),
                "wo": np.ascontiguousarray(Wo[512 * t:512 * (t + 1), :]),
                "lam": lam_arr,
                "rmsw": rmsw_arr,
                "masks": masks,
            })

    trace = bool(os.environ.get("DIFFATTN_TRACE"))
    res = run_bass_kernel_spmd(nc, in_maps, list(range(8)), trace=trace)
    kernel.last_results = res

    out = np.empty((B, length, HID), np.float32)
    for b in range(B):
        acc = res.results[4 * b]["part"].astype(np.float32).copy()
        for t in range(1, 4):
            acc += res.results[4 * b + t]["part"]
        out[b] = acc
    return out


# revision 12
# speedup vs baseline: 1.1419x; 1.1419x over previous
"""Differential-Transformer attention (DiffAttn) Trainium2 Bass kernel.

Sharding: 8 cores = 2 (batch) x 4 (head-group tensor parallel).
Core c = 4*b + t handles batch b, query heads 4t..4t+3, kv head t,
and the two v-heads its query heads need (t//2 and t//2+2).
o_proj is row-parallel: each core returns a partial [L, HID] product;
the host sums the 4 partials per batch (the "unshard" step).

All matmuls run as float32r (fp32 data, reduced-precision multiply,
full PE speed at moving-dim >= 256). Softmax is computed without
max-subtraction (score magnitudes are bounded ~5, exp is safe in fp32)
on transposed score tiles S^T[k, q] so that the AV matmul needs no
transposes. The softmax denominator and the RMS-norm sum-of-squares
are computed with all-ones stationary matmuls which replicate the
result across all 128 partitions, so the combined normalization scale
  comb = rsqrt(ssq + 128*eps*den^2)       (algebraically exact fold of
                                           1/den, RMS rsqrt, and eps)
is computed on full [128, 512] tiles; rsqrt is exp(-0.5*ln(r)) so the
scalar engine only ever uses one LUT table set (exp/ln/copy/square).
"""

import os
import sys

import numpy as np

for _p in ("/opt/trn_rl_repo",):
    if _p not in sys.path and os.path.isdir(_p):
        sys.path.insert(0, _p)

B = 2
L = 2048
HID = 2048
D = 128
H = 16
NH = 4            # query heads per core
CT = HID // 128   # contraction tiles for the projections
EPS = 1e-6
LAMBDA_INIT = 0.2

_CACHE = {}


def _build(length=L):
    from concourse import bacc
    import concourse.mybir as mybir
    import concourse.tile as tile

    f32 = mybir.dt.float32
    f32r = mybir.dt.float32r
    Act = mybir.ActivationFunctionType

    # All ACT functions used here (Exp, Ln, Square, Copy) live together in
    # the 'natural_log_exp_and_others' LUT set, but the table-load pass
    # assigns each activation the first set containing its function, which
    # alternates sets and inserts a ~1.3us table reload per switch (~42us).
    # Restrict the pass's view to that one set (keeping list positions so
    # act_func_set_id indices stay valid) -> exactly one load total.
    _orig_tables = bacc.get_activation_tables

    def _only_ln_exp(arch):
        t = _orig_tables(arch)
        keep = "natural_log_exp_and_others"
        if keep not in t:
            return t
        return {name: (s if name == keep else set()) for name, s in t.items()}

    bacc.get_activation_tables = _only_ln_exp
    try:
        return _build_inner(length, bacc, mybir, tile, f32, f32r, Act)
    finally:
        bacc.get_activation_tables = _orig_tables


def _build_inner(length, bacc, mybir, tile, f32, f32r, Act):
    NJ = length // 512    # q-slices
    NLB = length // 128   # l/k blocks

    nc = bacc.Bacc()
    hsT = nc.dram_tensor("hsT", [HID, length], f32, kind="ExternalInput")
    cosT = nc.dram_tensor("cosT", [D, length], f32, kind="ExternalInput")
    sinT = nc.dram_tensor("sinT", [D, length], f32, kind="ExternalInput")
    wq = nc.dram_tensor("wq", [HID, NH * D], f32, kind="ExternalInput")
    wk = nc.dram_tensor("wk", [HID, D], f32, kind="ExternalInput")
    wv = nc.dram_tensor("wv", [HID, 2 * D], f32, kind="ExternalInput")
    wo = nc.dram_tensor("wo", [NH * D, HID], f32, kind="ExternalInput")
    lam = nc.dram_tensor("lam", [D, 1], f32, kind="ExternalInput")
    rmsw = nc.dram_tensor("rmsw", [D, 1], f32, kind="ExternalInput")
    masks = nc.dram_tensor("masks", [4, D, 512], f32, kind="ExternalInput")
    part = nc.dram_tensor("part", [length, HID], f32, kind="ExternalOutput")

    inv_sqrt_d = 1.0 / np.sqrt(np.float32(D))

    with tile.TileContext(nc) as tc:
        with tc.tile_pool(name="persist", bufs=1) as persist:
            qT = persist.tile([D, NH, length], f32r, tag="qT")
            kT = persist.tile([D, length], f32r, tag="kT")
            veff = persist.tile([D, NLB, D], f32r, tag="veff")
            mask_t = persist.tile([D, 4, 512], f32r, tag="mask")
            lam_t = persist.tile([D, 1], f32, tag="lam")
            rmsw_t = persist.tile([D, 1], f32, tag="rmsw")
            ones_t = persist.tile([D, D], f32r, tag="ones")
            ones_stage = persist.tile([D, D], f32, tag="ones_stage")

            nc.sync.dma_start(out=lam_t, in_=lam[:, :])

            # ---------------- Phase B: projections + RoPE -----------------
            with tc.tile_pool(name="wpool", bufs=1) as wpool, \
                 tc.tile_pool(name="hspool", bufs=24) as hspool, \
                 tc.tile_pool(name="stpool", bufs=8) as stpool, \
                 tc.tile_pool(name="cspool", bufs=2) as cspool, \
                 tc.tile_pool(name="btmp", bufs=4) as btmp, \
                 tc.tile_pool(name="bpsum", bufs=3, space="PSUM") as bpsum, \
                 tc.tile_pool(name="vpsum", bufs=2, space="PSUM") as vpsum:

                wq_t = wpool.tile([D, CT, NH * D], f32r, tag="wq")
                wk_t = wpool.tile([D, CT, D], f32r, tag="wk")
                wv_t = wpool.tile([D, CT, 2 * D], f32r, tag="wv")
                wq_r = wq.rearrange("(c p) m -> p c m", p=D)
                wk_r = wk.rearrange("(c p) m -> p c m", p=D)
                wv_r = wv.rearrange("(c p) m -> p c m", p=D)
                def load_hs_chunk(c, j):
                    st = stpool.tile([D, 512], f32, tag="stage")
                    ch = hspool.tile([D, 512], f32r, tag="hs")
                    nc.sync.dma_start(
                        out=st, in_=hsT[128 * c:128 * (c + 1),
                                        512 * j:512 * (j + 1)])
                    nc.vector.tensor_copy(out=ch, in_=st)
                    return ch

                # interleave weight-chunk and first-slice hs-chunk loads so
                # the first projection matmul starts as soon as c=0 lands
                chunks0 = []
                for c in range(CT):
                    st = stpool.tile([D, 512], f32, tag="stage")
                    nc.sync.dma_start(out=st, in_=wq_r[:, c, :])
                    nc.vector.tensor_copy(out=wq_t[:, c, :], in_=st)
                    st = stpool.tile([D, 512], f32, tag="stage")
                    nc.sync.dma_start(out=st[:, 0:D], in_=wk_r[:, c, :])
                    nc.sync.dma_start(out=st[:, D:3 * D], in_=wv_r[:, c, :])
                    nc.vector.tensor_copy(out=wk_t[:, c, :], in_=st[:, 0:D])
                    nc.vector.tensor_copy(out=wv_t[:, c, :], in_=st[:, D:3 * D])
                    chunks0.append(load_hs_chunk(c, 0))

                for j in range(NJ):
                    sl = slice(512 * j, 512 * (j + 1))
                    cos_s = cspool.tile([D, 512], f32, tag="cos")
                    sin_s = cspool.tile([D, 512], f32, tag="sin")
                    nc.sync.dma_start(out=cos_s, in_=cosT[:, sl])
                    nc.sync.dma_start(out=sin_s, in_=sinT[:, sl])

                    chunks = chunks0 if j == 0 else [
                        load_hs_chunk(c, j) for c in range(CT)]

                    # q heads + k head in [D, l] (transposed) layout
                    for db in range(NH + 1):
                        ps = bpsum.tile([D, 512], f32, tag="qk")
                        for c in range(CT):
                            lhsT = (wq_t[:, c, 128 * db:128 * (db + 1)]
                                    if db < NH else wk_t[:, c, :])
                            nc.tensor.matmul(ps, lhsT, chunks[c],
                                             start=(c == 0), stop=(c == CT - 1))
                        # RoPE: rotate_half swaps partition halves; sin's two
                        # halves are identical (emb = concat([freqs, freqs])),
                        # so rot(q)*sin lands via partition-swapped reads of
                        # the PSUM tile (PSUM+SBUF input pairs are exempt from
                        # the same-base-partition rule).
                        t1 = btmp.tile([D, 512], f32, tag="t1")
                        t2 = btmp.tile([D, 512], f32, tag="t2")
                        nc.vector.tensor_mul(t1, ps, cos_s)
                        nc.vector.tensor_mul(t2[0:64, :], ps[64:128, :],
                                             sin_s[0:64, :])
                        nc.vector.tensor_mul(t2[64:128, :], ps[0:64, :],
                                             sin_s[64:128, :])
                        if db < NH:
                            dst_lo = qT[0:64, db, sl]
                            dst_hi = qT[64:128, db, sl]
                        else:
                            dst_lo = kT[0:64, sl]
                            dst_hi = kT[64:128, sl]
                        nc.vector.tensor_sub(dst_lo, t1[0:64, :], t2[0:64, :])
                        nc.vector.tensor_add(dst_hi, t1[64:128, :],
                                             t2[64:128, :])

                    # v in natural [l, d] layout; combine v_lo - lam*v_hi
                    for i in range(4):
                        kb = 4 * j + i
                        psv = vpsum.tile([D, 2 * D], f32, tag="v")
                        for c in range(CT):
                            nc.tensor.matmul(
                                psv, chunks[c][:, 128 * i:128 * (i + 1)],
                                wv_t[:, c, :],
                                start=(c == 0), stop=(c == CT - 1))
                        tv = btmp.tile([D, D], f32, tag="tv")
                        nc.vector.tensor_scalar_mul(tv, psv[:, D:2 * D], lam_t)
                        nc.vector.tensor_sub(veff[:, kb, :], psv[:, 0:D], tv)

            # ---------------- Phase C + D ---------------------------------
            with tc.tile_pool(name="cdpool", bufs=1) as cdpool, \
                 tc.tile_pool(name="wostage", bufs=4) as wostage:
                finalT = cdpool.tile([D, NH, length], f32r, tag="finalT")
                wo_t = cdpool.tile([D, NH, HID], f32r, tag="wo")

                nc.sync.dma_start(out=mask_t.bitcast(f32),
                                  in_=masks.rearrange("m p q -> p m q"))
                nc.sync.dma_start(out=rmsw_t, in_=rmsw[:, :])
                nc.vector.memset(ones_stage, 1.0)
                nc.vector.tensor_copy(out=ones_t, in_=ones_stage)

                # prefetch + round wo during the attention phase
                wo_r = wo.rearrange("(h p) m -> p h m", p=D)
                for h in range(NH):
                    for q4 in range(HID // 512):
                        st = wostage.tile([D, 512], f32, tag="wost")
                        nc.sync.dma_start(
                            out=st, in_=wo_r[:, h, 512 * q4:512 * (q4 + 1)])
                        nc.vector.tensor_copy(
                            out=wo_t[:, h, 512 * q4:512 * (q4 + 1)], in_=st)

                with tc.tile_pool(name="cpool", bufs=6) as cpool, \
                     tc.tile_pool(name="ctmp", bufs=3) as ctmp, \
                     tc.tile_pool(name="pso_p", bufs=2, space="PSUM") as pso_p, \
                     tc.tile_pool(name="pss_p", bufs=2, space="PSUM") as pss_p, \
                     tc.tile_pool(name="psd_p", bufs=2, space="PSUM") as psd_p, \
                     tc.tile_pool(name="psq_p", bufs=2, space="PSUM") as psq_p:
                    for j in range(NJ):
                        sl = slice(512 * j, 512 * (j + 1))
                        nkb = 4 * j + 4
                        for h in range(NH):
                            pso = pso_p.tile([D, 512], f32, tag="o")
                            psden = psd_p.tile([D, 512], f32, tag="den")
                            # software-pipelined: emit score(kb+1) before
                            # den/AV(kb) so PE never waits on ACT's exp
                            se_l = {}

                            def score(kb):
                                pss = pss_p.tile([D, 512], f32, tag="s")
                                nc.tensor.matmul(
                                    pss, kT[:, 128 * kb:128 * (kb + 1)],
                                    qT[:, h, sl], start=True, stop=True)
                                se = cpool.tile([D, 512], f32r, tag="se")
                                nc.scalar.activation(se, pss, Act.Exp,
                                                     scale=float(inv_sqrt_d))
                                if kb >= 4 * j:
                                    nc.vector.tensor_mul(
                                        se, se, mask_t[:, kb - 4 * j, :])
                                se_l[kb] = se

                            score(0)
                            for kb in range(nkb):
                                if kb + 1 < nkb:
                                    score(kb + 1)
                                se = se_l.pop(kb)
                                nc.tensor.matmul(psden, ones_t, se,
                                                 start=(kb == 0),
                                                 stop=(kb == nkb - 1))
                                nc.tensor.matmul(pso, veff[:, kb, :], se,
                                                 start=(kb == 0),
                                                 stop=(kb == nkb - 1))

                            # copy u out early so the psum slot frees without
                            # waiting on the comb chain
                            u_f = cpool.tile([D, 512], f32, tag="uf")
                            nc.scalar.copy(u_f, pso)
                            # comb = rsqrt(ssq + 128*eps*den^2), all [128,512]
                            sq = cpool.tile([D, 512], f32r, tag="sq")
                            nc.vector.tensor_mul(sq, u_f, u_f)
                            psss = psq_p.tile([D, 512], f32, tag="ss")
                            nc.tensor.matmul(psss, ones_t, sq,
                                             start=True, stop=True)
                            c1 = ctmp.tile([D, 512], f32, tag="c1")
                            nc.scalar.activation(c1, psden, Act.Square)
                            nc.vector.tensor_scalar_mul(c1, c1, 128.0 * EPS)
                            nc.vector.tensor_add(c1, c1, psss)
                            lnr = ctmp.tile([D, 512], f32, tag="lnr")
                            nc.scalar.activation(lnr, c1, Act.Ln)
                            comb = ctmp.tile([D, 512], f32, tag="comb")
                            nc.scalar.activation(comb, lnr, Act.Exp, scale=-0.5)
                            ft = ctmp.tile([D, 512], f32, tag="ft")
                            nc.vector.tensor_mul(ft, u_f, comb)
                            nc.vector.tensor_scalar_mul(
                                finalT[:, h, sl], ft, rmsw_t)

                # ---------------- Phase D: o_proj ------------------------
                with tc.tile_pool(name="dout", bufs=4) as dout, \
                     tc.tile_pool(name="dpsum", bufs=2, space="PSUM") as dpsum:
                    for lb in range(NLB):
                        lsl = slice(128 * lb, 128 * (lb + 1))
                        for osl in range(HID // 512):
                            osl_s = slice(512 * osl, 512 * (osl + 1))
                            ps = dpsum.tile([D, 512], f32, tag="op")
                            for h in range(NH):
                                nc.tensor.matmul(ps, finalT[:, h, lsl],
                                                 wo_t[:, h, osl_s],
                                                 start=(h == 0), stop=(h == NH - 1))
                            ob = dout.tile([D, 512], f32, tag="ob")
                            nc.scalar.copy(ob, ps)
                            nc.sync.dma_start(out=part[lsl, osl_s], in_=ob)

    nc.finalize()
    return nc


def _causal_masks():
    m = np.zeros((4, D, 512), np.float32)
    for r in range(4):
        for p in range(D):
            q0 = 128 * r + p
            if q0 < 512:
                m[r, p, q0:] = 1.0
    return m


def kernel(hidden_states, cos, sin, Wq, Wk, Wv, Wo,
           lambda_q1, lambda_k1, lambda_q2, lambda_k2, rms_weight):
    from concourse.bass_utils import run_bass_kernel_spmd

    length = hidden_states.shape[1]
    if length not in _CACHE:
        _CACHE[length] = _build(length)
    nc = _CACHE[length]

    hidden_states = np.asarray(hidden_states, np.float32)
    cos = np.asarray(cos, np.float32)
    sin = np.asarray(sin, np.float32)

    lam_full = np.float32(
        np.exp(np.float32(np.dot(np.asarray(lambda_q1, np.float32),
                                 np.asarray(lambda_k1, np.float32)))
               + np.float32(np.dot(np.asarray(lambda_q2, np.float32),
                                   np.asarray(lambda_k2, np.float32))))
        + np.float32(LAMBDA_INIT))
    lam_arr = np.full((D, 1), lam_full, np.float32)
    rmsw_arr = np.ascontiguousarray(
        (np.asarray(rms_weight, np.float32)
         * np.float32((1.0 - LAMBDA_INIT) * np.sqrt(128.0))).reshape(D, 1))
    masks = _causal_masks()

    Wq = np.asarray(Wq, np.float32)
    Wk = np.asarray(Wk, np.float32)
    Wv = np.asarray(Wv, np.float32)
    Wo = np.asarray(Wo, np.float32)

    in_maps = []
    for b in range(B):
        hsT_b = np.ascontiguousarray(hidden_states[b].T)
        cosT_b = np.ascontiguousarray(cos[b].T)
        sinT_b = np.ascontiguousarray(sin[b].T)
        for t in range(4):
            vlo, vhi = t // 2, t // 2 + 2
            in_maps.append({
                "hsT": hsT_b,
                "cosT": cosT_b,
                "sinT": sinT_b,
                "wq": np.ascontiguousarray(Wq[:, 512 * t:512 * (t + 1)]),
                "wk": np.ascontiguousarray(Wk[:, 128 * t:128 * (t + 1)]),
                "wv": np.ascontiguousarray(np.concatenate(
                    [Wv[:, 128 * vlo:128 * (vlo + 1)],
                     Wv[:, 128 * vhi:128 * (vhi + 1)]], axis=1)),
                "wo": np.ascontiguousarray(Wo[512 * t:512 * (t + 1), :]),
                "lam": lam_arr,
                "rmsw": rmsw_arr,
                "masks": masks,
            })

    trace = bool(os.environ.get("DIFFATTN_TRACE"))
    res = run_bass_kernel_spmd(nc, in_maps, list(range(8)), trace=trace)
    kernel.last_results = res

    out = np.empty((B, length, HID), np.float32)
    for b in range(B):
        acc = res.results[4 * b]["part"].astype(np.float32).copy()
        for t in range(1, 4):
            acc += res.results[4 * b + t]["part"]
        out[b] = acc
    return out


# revision 13
# speedup vs baseline: 1.2227x; 1.0707x over previous
"""Differential-Transformer attention (DiffAttn) Trainium2 Bass kernel.

Sharding: 8 cores = 2 (batch) x 4 (head-group tensor parallel).
Core c = 4*b + t handles batch b, query heads 4t..4t+3, kv head t,
and the two v-heads its query heads need (t//2 and t//2+2).
o_proj is row-parallel: each core returns a partial [L, HID] product;
the host sums the 4 partials per batch (the "unshard" step).

All matmuls run as float32r (fp32 data, reduced-precision multiply,
full PE speed at moving-dim >= 256). Softmax is computed without
max-subtraction (score magnitudes are bounded ~5, exp is safe in fp32)
on transposed score tiles S^T[k, q] so that the AV matmul needs no
transposes. The softmax denominator and the RMS-norm sum-of-squares
are computed with all-ones stationary matmuls which replicate the
result across all 128 partitions, so the combined normalization scale
  comb = rsqrt(ssq + 128*eps*den^2)       (algebraically exact fold of
                                           1/den, RMS rsqrt, and eps)
is computed on full [128, 512] tiles; rsqrt is exp(-0.5*ln(r)) so the
scalar engine only ever uses one LUT table set (exp/ln/copy/square).
"""

import os
import sys

import numpy as np

for _p in ("/opt/trn_rl_repo",):
    if _p not in sys.path and os.path.isdir(_p):
        sys.path.insert(0, _p)

B = 2
L = 2048
HID = 2048
D = 128
H = 16
NH = 4            # query heads per core
CT = HID // 128   # contraction tiles for the projections
EPS = 1e-6
LAMBDA_INIT = 0.2

_CACHE = {}


def _build(length=L):
    from concourse import bacc
    import concourse.mybir as mybir
    import concourse.tile as tile

    f32 = mybir.dt.float32
    f32r = mybir.dt.float32r
    Act = mybir.ActivationFunctionType

    # All ACT functions used here (Exp, Ln, Square, Copy) live together in
    # the 'natural_log_exp_and_others' LUT set, but the table-load pass
    # assigns each activation the first set containing its function, which
    # alternates sets and inserts a ~1.3us table reload per switch (~42us).
    # Restrict the pass's view to that one set (keeping list positions so
    # act_func_set_id indices stay valid) -> exactly one load total.
    _orig_tables = bacc.get_activation_tables

    def _only_ln_exp(arch):
        t = _orig_tables(arch)
        keep = "natural_log_exp_and_others"
        if keep not in t:
            return t
        return {name: (s if name == keep else set()) for name, s in t.items()}

    bacc.get_activation_tables = _only_ln_exp
    try:
        return _build_inner(length, bacc, mybir, tile, f32, f32r, Act)
    finally:
        bacc.get_activation_tables = _orig_tables


def _build_inner(length, bacc, mybir, tile, f32, f32r, Act):
    NJ = length // 512    # q-slices
    NLB = length // 128   # l/k blocks

    nc = bacc.Bacc()
    hsT = nc.dram_tensor("hsT", [HID, length], f32, kind="ExternalInput")
    cosT = nc.dram_tensor("cosT", [D, length], f32, kind="ExternalInput")
    sinT = nc.dram_tensor("sinT", [D, length], f32, kind="ExternalInput")
    wq = nc.dram_tensor("wq", [HID, NH * D], f32, kind="ExternalInput")
    wk = nc.dram_tensor("wk", [HID, D], f32, kind="ExternalInput")
    wv = nc.dram_tensor("wv", [HID, 2 * D], f32, kind="ExternalInput")
    wo = nc.dram_tensor("wo", [NH * D, HID], f32, kind="ExternalInput")
    lam = nc.dram_tensor("lam", [D, 1], f32, kind="ExternalInput")
    rmsw = nc.dram_tensor("rmsw", [D, 1], f32, kind="ExternalInput")
    masks = nc.dram_tensor("masks", [4, D, 512], f32, kind="ExternalInput")
    part = nc.dram_tensor("part", [length, HID], f32, kind="ExternalOutput")

    inv_sqrt_d = 1.0 / np.sqrt(np.float32(D))

    with tile.TileContext(nc) as tc:
        with tc.tile_pool(name="persist", bufs=1) as persist:
            qT = persist.tile([D, NH, length], f32r, tag="qT")
            kT = persist.tile([D, length], f32r, tag="kT")
            veff = persist.tile([D, NLB, D], f32r, tag="veff")
            mask_t = persist.tile([D, 4, 512], f32r, tag="mask")
            lam_t = persist.tile([D, 1], f32, tag="lam")
            rmsw_t = persist.tile([D, 1], f32, tag="rmsw")
            ones_t = persist.tile([D, D], f32r, tag="ones")
            ones_stage = persist.tile([D, D], f32, tag="ones_stage")

            nc.sync.dma_start(out=lam_t, in_=lam[:, :])

            # ---------------- Phase B: projections + RoPE -----------------
            with tc.tile_pool(name="wpool", bufs=1) as wpool, \
                 tc.tile_pool(name="hspool", bufs=24) as hspool, \
                 tc.tile_pool(name="stpool", bufs=8) as stpool, \
                 tc.tile_pool(name="cspool", bufs=2) as cspool, \
                 tc.tile_pool(name="btmp", bufs=4) as btmp, \
                 tc.tile_pool(name="bpsum", bufs=3, space="PSUM") as bpsum, \
                 tc.tile_pool(name="vpsum", bufs=2, space="PSUM") as vpsum:

                wq_t = wpool.tile([D, CT, NH * D], f32r, tag="wq")
                wk_t = wpool.tile([D, CT, D], f32r, tag="wk")
                wv_t = wpool.tile([D, CT, 2 * D], f32r, tag="wv")
                wq_r = wq.rearrange("(c p) m -> p c m", p=D)
                wk_r = wk.rearrange("(c p) m -> p c m", p=D)
                wv_r = wv.rearrange("(c p) m -> p c m", p=D)
                def load_hs_chunk(c, j):
                    st = stpool.tile([D, 512], f32, tag="stage")
                    ch = hspool.tile([D, 512], f32r, tag="hs")
                    nc.sync.dma_start(
                        out=st, in_=hsT[128 * c:128 * (c + 1),
                                        512 * j:512 * (j + 1)])
                    nc.vector.tensor_copy(out=ch, in_=st)
                    return ch

                # interleave weight-chunk and first-slice hs-chunk loads so
                # the first projection matmul starts as soon as c=0 lands
                chunks0 = []
                for c in range(CT):
                    st = stpool.tile([D, 512], f32, tag="stage")
                    nc.sync.dma_start(out=st, in_=wq_r[:, c, :])
                    nc.vector.tensor_copy(out=wq_t[:, c, :], in_=st)
                    st = stpool.tile([D, 512], f32, tag="stage")
                    nc.sync.dma_start(out=st[:, 0:D], in_=wk_r[:, c, :])
                    nc.sync.dma_start(out=st[:, D:3 * D], in_=wv_r[:, c, :])
                    nc.vector.tensor_copy(out=wk_t[:, c, :], in_=st[:, 0:D])
                    nc.vector.tensor_copy(out=wv_t[:, c, :], in_=st[:, D:3 * D])
                    chunks0.append(load_hs_chunk(c, 0))

                for j in range(NJ):
                    sl = slice(512 * j, 512 * (j + 1))
                    cos_s = cspool.tile([D, 512], f32, tag="cos")
                    sin_s = cspool.tile([D, 512], f32, tag="sin")
                    nc.sync.dma_start(out=cos_s, in_=cosT[:, sl])
                    nc.sync.dma_start(out=sin_s, in_=sinT[:, sl])

                    chunks = chunks0 if j == 0 else [
                        load_hs_chunk(c, j) for c in range(CT)]

                    # q heads + k head in [D, l] (transposed) layout
                    for db in range(NH + 1):
                        ps = bpsum.tile([D, 512], f32, tag="qk")
                        for c in range(CT):
                            lhsT = (wq_t[:, c, 128 * db:128 * (db + 1)]
                                    if db < NH else wk_t[:, c, :])
                            nc.tensor.matmul(ps, lhsT, chunks[c],
                                             start=(c == 0), stop=(c == CT - 1))
                        # RoPE: rotate_half swaps partition halves; sin's two
                        # halves are identical (emb = concat([freqs, freqs])),
                        # so rot(q)*sin lands via partition-swapped reads of
                        # the PSUM tile (PSUM+SBUF input pairs are exempt from
                        # the same-base-partition rule).
                        t1 = btmp.tile([D, 512], f32, tag="t1")
                        t2 = btmp.tile([D, 512], f32, tag="t2")
                        nc.vector.tensor_mul(t1, ps, cos_s)
                        nc.vector.tensor_mul(t2[0:64, :], ps[64:128, :],
                                             sin_s[0:64, :])
                        nc.vector.tensor_mul(t2[64:128, :], ps[0:64, :],
                                             sin_s[64:128, :])
                        if db < NH:
                            dst_lo = qT[0:64, db, sl]
                            dst_hi = qT[64:128, db, sl]
                        else:
                            dst_lo = kT[0:64, sl]
                            dst_hi = kT[64:128, sl]
                        nc.vector.tensor_sub(dst_lo, t1[0:64, :], t2[0:64, :])
                        nc.vector.tensor_add(dst_hi, t1[64:128, :],
                                             t2[64:128, :])

                    # v in natural [l, d] layout; combine v_lo - lam*v_hi
                    for i in range(4):
                        kb = 4 * j + i
                        psv = vpsum.tile([D, 2 * D], f32, tag="v")
                        for c in range(CT):
                            nc.tensor.matmul(
                                psv, chunks[c][:, 128 * i:128 * (i + 1)],
                                wv_t[:, c, :],
                                start=(c == 0), stop=(c == CT - 1))
                        tv = btmp.tile([D, D], f32, tag="tv")
                        nc.vector.tensor_scalar_mul(tv, psv[:, D:2 * D], lam_t)
                        nc.vector.tensor_sub(veff[:, kb, :], psv[:, 0:D], tv)

            # ---------------- Phase C + D ---------------------------------
            with tc.tile_pool(name="cdpool", bufs=1) as cdpool, \
                 tc.tile_pool(name="wostage", bufs=4) as wostage:
                finalT = cdpool.tile([D, NH, length], f32r, tag="finalT")
                wo_t = cdpool.tile([D, NH, HID], f32r, tag="wo")

                nc.sync.dma_start(out=mask_t.bitcast(f32),
                                  in_=masks.rearrange("m p q -> p m q"))
                nc.sync.dma_start(out=rmsw_t, in_=rmsw[:, :])
                nc.vector.memset(ones_stage, 1.0)
                nc.vector.tensor_copy(out=ones_t, in_=ones_stage)

                # prefetch + round wo during the attention phase
                wo_r = wo.rearrange("(h p) m -> p h m", p=D)
                for h in range(NH):
                    for q4 in range(HID // 512):
                        st = wostage.tile([D, 512], f32, tag="wost")
                        nc.sync.dma_start(
                            out=st, in_=wo_r[:, h, 512 * q4:512 * (q4 + 1)])
                        nc.vector.tensor_copy(
                            out=wo_t[:, h, 512 * q4:512 * (q4 + 1)], in_=st)

                with tc.tile_pool(name="cpool", bufs=6) as cpool, \
                     tc.tile_pool(name="ctmp", bufs=3) as ctmp, \
                     tc.tile_pool(name="pso_p", bufs=1, space="PSUM") as pso_p, \
                     tc.tile_pool(name="pss_p", bufs=2, space="PSUM") as pss_p, \
                     tc.tile_pool(name="psd_p", bufs=2, space="PSUM") as psd_p, \
                     tc.tile_pool(name="psq_p", bufs=1, space="PSUM") as psq_p:
                    for j in range(NJ):
                        sl = slice(512 * j, 512 * (j + 1))
                        nkb = 4 * j + 4
                        npair = nkb // 2
                        for h in range(NH):
                            pso = pso_p.tile([D, 512], f32, tag="o")
                            psden = psd_p.tile([D, 512], f32, tag="den")
                            # kb tiles processed in pairs: both score matmuls
                            # write one 2-bank psum tile, one Exp covers
                            # [128, 1024], diagonal masks applied pairwise.
                            # Emit pair t+1's scores before pair t's den/AV so
                            # PE never waits on ACT's exp.
                            se_l = {}

                            def score_pair(t):
                                pss = pss_p.tile([D, 2, 512], f32, tag="s")
                                for i in (0, 1):
                                    kb = 2 * t + i
                                    nc.tensor.matmul(
                                        pss[:, i, :],
                                        kT[:, 128 * kb:128 * (kb + 1)],
                                        qT[:, h, sl], start=True, stop=True)
                                se = cpool.tile([D, 2, 512], f32r, tag="se")
                                nc.scalar.activation(se, pss, Act.Exp,
                                                     scale=float(inv_sqrt_d))
                                if t == 2 * j:
                                    nc.vector.tensor_mul(
                                        se, se, mask_t[:, 0:2, :])
                                elif t == 2 * j + 1:
                                    nc.vector.tensor_mul(
                                        se, se, mask_t[:, 2:4, :])
                                se_l[t] = se

                            score_pair(0)
                            for t in range(npair):
                                if t + 1 < npair:
                                    score_pair(t + 1)
                                se = se_l.pop(t)
                                for i in (0, 1):
                                    kb = 2 * t + i
                                    last = kb == nkb - 1
                                    nc.tensor.matmul(psden, ones_t, se[:, i, :],
                                                     start=(kb == 0),
                                                     stop=last)
                                    nc.tensor.matmul(pso, veff[:, kb, :],
                                                     se[:, i, :],
                                                     start=(kb == 0),
                                                     stop=last)

                            # copy u out early so the psum slot frees without
                            # waiting on the comb chain
                            u_f = cpool.tile([D, 512], f32, tag="uf")
                            nc.scalar.copy(u_f, pso)
                            # comb = rsqrt(ssq + 128*eps*den^2), all [128,512]
                            sq = cpool.tile([D, 512], f32r, tag="sq")
                            nc.vector.tensor_mul(sq, u_f, u_f)
                            psss = psq_p.tile([D, 512], f32, tag="ss")
                            nc.tensor.matmul(psss, ones_t, sq,
                                             start=True, stop=True)
                            c1 = ctmp.tile([D, 512], f32, tag="c1")
                            nc.scalar.activation(c1, psden, Act.Square)
                            nc.vector.tensor_scalar_mul(c1, c1, 128.0 * EPS)
                            nc.vector.tensor_add(c1, c1, psss)
                            lnr = ctmp.tile([D, 512], f32, tag="lnr")
                            nc.scalar.activation(lnr, c1, Act.Ln)
                            comb = ctmp.tile([D, 512], f32, tag="comb")
                            nc.scalar.activation(comb, lnr, Act.Exp, scale=-0.5)
                            ft = ctmp.tile([D, 512], f32, tag="ft")
                            nc.vector.tensor_mul(ft, u_f, comb)
                            nc.vector.tensor_scalar_mul(
                                finalT[:, h, sl], ft, rmsw_t)

                # ---------------- Phase D: o_proj ------------------------
                with tc.tile_pool(name="dout", bufs=4) as dout, \
                     tc.tile_pool(name="dpsum", bufs=2, space="PSUM") as dpsum:
                    for lb in range(NLB):
                        lsl = slice(128 * lb, 128 * (lb + 1))
                        for osl in range(HID // 512):
                            osl_s = slice(512 * osl, 512 * (osl + 1))
                            ps = dpsum.tile([D, 512], f32, tag="op")
                            for h in range(NH):
                                nc.tensor.matmul(ps, finalT[:, h, lsl],
                                                 wo_t[:, h, osl_s],
                                                 start=(h == 0), stop=(h == NH - 1))
                            ob = dout.tile([D, 512], f32, tag="ob")
                            nc.scalar.copy(ob, ps)
                            nc.sync.dma_start(out=part[lsl, osl_s], in_=ob)

    nc.finalize()
    return nc


def _causal_masks():
    m = np.zeros((4, D, 512), np.float32)
    for r in range(4):
        for p in range(D):
            q0 = 128 * r + p
            if q0 < 512:
                m[r, p, q0:] = 1.0
    return m


def kernel(hidden_states, cos, sin, Wq, Wk, Wv, Wo,
           lambda_q1, lambda_k1, lambda_q2, lambda_k2, rms_weight):
    from concourse.bass_utils import run_bass_kernel_spmd

    length = hidden_states.shape[1]
    if length not in _CACHE:
        _CACHE[length] = _build(length)
    nc = _CACHE[length]

    hidden_states = np.asarray(hidden_states, np.float32)
    cos = np.asarray(cos, np.float32)
    sin = np.asarray(sin, np.float32)

    lam_full = np.float32(
        np.exp(np.float32(np.dot(np.asarray(lambda_q1, np.float32),
                                 np.asarray(lambda_k1, np.float32)))
               + np.float32(np.dot(np.asarray(lambda_q2, np.float32),
                                   np.asarray(lambda_k2, np.float32))))
        + np.float32(LAMBDA_INIT))
    lam_arr = np.full((D, 1), lam_full, np.float32)
    rmsw_arr = np.ascontiguousarray(
        (np.asarray(rms_weight, np.float32)
         * np.float32((1.0 - LAMBDA_INIT) * np.sqrt(128.0))).reshape(D, 1))
    masks = _causal_masks()

    Wq = np.asarray(Wq, np.float32)
    Wk = np.asarray(Wk, np.float32)
    Wv = np.asarray(Wv, np.float32)
    Wo = np.asarray(Wo, np.float32)

    in_maps = []
    for b in range(B):
        hsT_b = np.ascontiguousarray(hidden_states[b].T)
        cosT_b = np.ascontiguousarray(cos[b].T)
        sinT_b = np.ascontiguousarray(sin[b].T)
        for t in range(4):
            vlo, vhi = t // 2, t // 2 + 2
            in_maps.append({
                "hsT": hsT_b,
                "cosT": cosT_b,
                "sinT": sinT_b,
                "wq": np.ascontiguousarray(Wq[:, 512 * t:512 * (t + 1)]),
                "wk": np.ascontiguousarray(Wk[:, 128 * t:128 * (t + 1)]),
                "wv": np.ascontiguousarray(np.concatenate(
                    [Wv[:, 128 * vlo:128 * (vlo + 1)],
                     Wv[:, 128 * vhi:128 * (vhi + 1)]], axis=1)),
                "wo": np.ascontiguousarray(Wo[512 * t:512 * (t + 1), :]),
                "lam": lam_arr,
                "rmsw": rmsw_arr,
                "masks": masks,
            })

    trace = bool(os.environ.get("DIFFATTN_TRACE"))
    res = run_bass_kernel_spmd(nc, in_maps, list(range(8)), trace=trace)
    kernel.last_results = res

    out = np.empty((B, length, HID), np.float32)
    for b in range(B):
        acc = res.results[4 * b]["part"].astype(np.float32).copy()
        for t in range(1, 4):
            acc += res.results[4 * b + t]["part"]
        out[b] = acc
    return out


# revision 14
# speedup vs baseline: 1.3319x; 1.0894x over previous
"""Differential-Transformer attention (DiffAttn) Trainium2 Bass kernel.

Sharding: 8 cores = 2 (batch) x 4 (head-group tensor parallel).
Core c = 4*b + t handles batch b, query heads 4t..4t+3, kv head t,
and the two v-heads its query heads need (t//2 and t//2+2).
o_proj is row-parallel: each core returns a partial [L, HID] product;
the host sums the 4 partials per batch (the "unshard" step).

All matmuls run as float32r (fp32 data, reduced-precision multiply,
full PE speed at moving-dim >= 256). Softmax is computed without
max-subtraction (score magnitudes are bounded ~5, exp is safe in fp32)
on transposed score tiles S^T[k, q] so that the AV matmul needs no
transposes. The softmax denominator and the RMS-norm sum-of-squares
are computed with all-ones stationary matmuls which replicate the
result across all 128 partitions, so the combined normalization scale
  comb = rsqrt(ssq + 128*eps*den^2)       (algebraically exact fold of
                                           1/den, RMS rsqrt, and eps)
is computed on full [128, 512] tiles; rsqrt is exp(-0.5*ln(r)) so the
scalar engine only ever uses one LUT table set (exp/ln/copy/square).
"""

import os
import sys

import numpy as np

for _p in ("/opt/trn_rl_repo",):
    if _p not in sys.path and os.path.isdir(_p):
        sys.path.insert(0, _p)

B = 2
L = 2048
HID = 2048
D = 128
H = 16
NH = 4            # query heads per core
CT = HID // 128   # contraction tiles for the projections
EPS = 1e-6
LAMBDA_INIT = 0.2

_CACHE = {}


def _build(length=L):
    from concourse import bacc
    import concourse.mybir as mybir
    import concourse.tile as tile

    f32 = mybir.dt.float32
    f32r = mybir.dt.float32r
    Act = mybir.ActivationFunctionType

    # All ACT functions used here (Exp, Ln, Square, Copy) live together in
    # the 'natural_log_exp_and_others' LUT set, but the table-load pass
    # assigns each activation the first set containing its function, which
    # alternates sets and inserts a ~1.3us table reload per switch (~42us).
    # Restrict the pass's view to that one set (keeping list positions so
    # act_func_set_id indices stay valid) -> exactly one load total.
    _orig_tables = bacc.get_activation_tables

    def _only_ln_exp(arch):
        t = _orig_tables(arch)
        keep = "natural_log_exp_and_others"
        if keep not in t:
            return t
        return {name: (s if name == keep else set()) for name, s in t.items()}

    bacc.get_activation_tables = _only_ln_exp
    try:
        return _build_inner(length, bacc, mybir, tile, f32, f32r, Act)
    finally:
        bacc.get_activation_tables = _orig_tables


def _build_inner(length, bacc, mybir, tile, f32, f32r, Act):
    NJ = length // 512    # q-slices
    NLB = length // 128   # l/k blocks

    nc = bacc.Bacc()
    hsT = nc.dram_tensor("hsT", [HID, length], f32, kind="ExternalInput")
    cosT = nc.dram_tensor("cosT", [D, length], f32, kind="ExternalInput")
    sinT = nc.dram_tensor("sinT", [D, length], f32, kind="ExternalInput")
    wq = nc.dram_tensor("wq", [HID, NH * D], f32, kind="ExternalInput")
    wk = nc.dram_tensor("wk", [HID, D], f32, kind="ExternalInput")
    wv = nc.dram_tensor("wv", [HID, 2 * D], f32, kind="ExternalInput")
    wo = nc.dram_tensor("wo", [NH * D, HID], f32, kind="ExternalInput")
    lam = nc.dram_tensor("lam", [D, 1], f32, kind="ExternalInput")
    rmsw = nc.dram_tensor("rmsw", [D, 1], f32, kind="ExternalInput")
    masks = nc.dram_tensor("masks", [4, D, 512], f32, kind="ExternalInput")
    part = nc.dram_tensor("part", [length, HID], f32, kind="ExternalOutput")

    inv_sqrt_d = 1.0 / np.sqrt(np.float32(D))

    with tile.TileContext(nc) as tc:
        with tc.tile_pool(name="persist", bufs=1) as persist:
            qT = persist.tile([D, NH, length], f32r, tag="qT")
            kT = persist.tile([D, length], f32r, tag="kT")
            veff = persist.tile([D, NLB, D], f32r, tag="veff")
            mask_t = persist.tile([D, 4, 512], f32r, tag="mask")
            lam_t = persist.tile([D, 1], f32, tag="lam")
            rmsw_t = persist.tile([D, 1], f32, tag="rmsw")
            ones_t = persist.tile([D, D], f32r, tag="ones")
            ones_stage = persist.tile([D, D], f32, tag="ones_stage")

            nc.sync.dma_start(out=lam_t, in_=lam[:, :])

            # ---------------- Phase B: projections + RoPE -----------------
            with tc.tile_pool(name="wpool", bufs=1) as wpool, \
                 tc.tile_pool(name="hspool", bufs=24) as hspool, \
                 tc.tile_pool(name="stpool", bufs=8) as stpool, \
                 tc.tile_pool(name="cspool", bufs=2) as cspool, \
                 tc.tile_pool(name="btmp", bufs=4) as btmp, \
                 tc.tile_pool(name="bpsum", bufs=3, space="PSUM") as bpsum, \
                 tc.tile_pool(name="vpsum", bufs=2, space="PSUM") as vpsum:

                wq_t = wpool.tile([D, CT, NH * D], f32r, tag="wq")
                wk_t = wpool.tile([D, CT, D], f32r, tag="wk")
                wv_t = wpool.tile([D, CT, 2 * D], f32r, tag="wv")
                wq_r = wq.rearrange("(c p) m -> p c m", p=D)
                wk_r = wk.rearrange("(c p) m -> p c m", p=D)
                wv_r = wv.rearrange("(c p) m -> p c m", p=D)
                def load_hs_chunk(c, j):
                    st = stpool.tile([D, 512], f32, tag="stage")
                    ch = hspool.tile([D, 512], f32r, tag="hs")
                    nc.sync.dma_start(
                        out=st, in_=hsT[128 * c:128 * (c + 1),
                                        512 * j:512 * (j + 1)])
                    nc.vector.tensor_copy(out=ch, in_=st)
                    return ch

                # interleave weight-chunk and first-slice hs-chunk loads so
                # the first projection matmul starts as soon as c=0 lands
                chunks0 = []
                for c in range(CT):
                    st = stpool.tile([D, 512], f32, tag="stage")
                    nc.sync.dma_start(out=st, in_=wq_r[:, c, :])
                    nc.vector.tensor_copy(out=wq_t[:, c, :], in_=st)
                    st = stpool.tile([D, 512], f32, tag="stage")
                    nc.sync.dma_start(out=st[:, 0:D], in_=wk_r[:, c, :])
                    nc.sync.dma_start(out=st[:, D:3 * D], in_=wv_r[:, c, :])
                    nc.vector.tensor_copy(out=wk_t[:, c, :], in_=st[:, 0:D])
                    nc.vector.tensor_copy(out=wv_t[:, c, :], in_=st[:, D:3 * D])
                    chunks0.append(load_hs_chunk(c, 0))

                for j in range(NJ):
                    sl = slice(512 * j, 512 * (j + 1))
                    cos_s = cspool.tile([D, 512], f32, tag="cos")
                    sin_s = cspool.tile([D, 512], f32, tag="sin")
                    nc.sync.dma_start(out=cos_s, in_=cosT[:, sl])
                    nc.sync.dma_start(out=sin_s, in_=sinT[:, sl])

                    chunks = chunks0 if j == 0 else [
                        load_hs_chunk(c, j) for c in range(CT)]

                    # q heads + k head in [D, l] (transposed) layout
                    for db in range(NH + 1):
                        ps = bpsum.tile([D, 512], f32, tag="qk")
                        for c in range(CT):
                            lhsT = (wq_t[:, c, 128 * db:128 * (db + 1)]
                                    if db < NH else wk_t[:, c, :])
                            nc.tensor.matmul(ps, lhsT, chunks[c],
                                             start=(c == 0), stop=(c == CT - 1))
                        # RoPE: rotate_half swaps partition halves; sin's two
                        # halves are identical (emb = concat([freqs, freqs])),
                        # so rot(q)*sin lands via partition-swapped reads of
                        # the PSUM tile (PSUM+SBUF input pairs are exempt from
                        # the same-base-partition rule).
                        t1 = btmp.tile([D, 512], f32, tag="t1")
                        t2 = btmp.tile([D, 512], f32, tag="t2")
                        nc.vector.tensor_mul(t1, ps, cos_s)
                        nc.vector.tensor_mul(t2[0:64, :], ps[64:128, :],
                                             sin_s[0:64, :])
                        nc.vector.tensor_mul(t2[64:128, :], ps[0:64, :],
                                             sin_s[64:128, :])
                        if db < NH:
                            dst_lo = qT[0:64, db, sl]
                            dst_hi = qT[64:128, db, sl]
                        else:
                            dst_lo = kT[0:64, sl]
                            dst_hi = kT[64:128, sl]
                        nc.vector.tensor_sub(dst_lo, t1[0:64, :], t2[0:64, :])
                        nc.vector.tensor_add(dst_hi, t1[64:128, :],
                                             t2[64:128, :])

                    # v in natural [l, d] layout; combine v_lo - lam*v_hi
                    for i in range(4):
                        kb = 4 * j + i
                        psv = vpsum.tile([D, 2 * D], f32, tag="v")
                        for c in range(CT):
                            nc.tensor.matmul(
                                psv, chunks[c][:, 128 * i:128 * (i + 1)],
                                wv_t[:, c, :],
                                start=(c == 0), stop=(c == CT - 1))
                        tv = btmp.tile([D, D], f32, tag="tv")
                        nc.vector.tensor_scalar_mul(tv, psv[:, D:2 * D], lam_t)
                        nc.vector.tensor_sub(veff[:, kb, :], psv[:, 0:D], tv)

            # ---------------- Phase C + D ---------------------------------
            with tc.tile_pool(name="cdpool", bufs=1) as cdpool, \
                 tc.tile_pool(name="wostage", bufs=4) as wostage:
                finalT = cdpool.tile([D, NH, length], f32r, tag="finalT")
                wo_t = cdpool.tile([D, NH, HID], f32r, tag="wo")

                nc.sync.dma_start(out=mask_t.bitcast(f32),
                                  in_=masks.rearrange("m p q -> p m q"))
                nc.sync.dma_start(out=rmsw_t, in_=rmsw[:, :])
                nc.vector.memset(ones_stage, 1.0)
                nc.vector.tensor_copy(out=ones_t, in_=ones_stage)

                # prefetch + round wo during the attention phase
                wo_r = wo.rearrange("(h p) m -> p h m", p=D)
                for h in range(NH):
                    for q4 in range(HID // 512):
                        st = wostage.tile([D, 512], f32, tag="wost")
                        nc.sync.dma_start(
                            out=st, in_=wo_r[:, h, 512 * q4:512 * (q4 + 1)])
                        nc.vector.tensor_copy(
                            out=wo_t[:, h, 512 * q4:512 * (q4 + 1)], in_=st)

                with tc.tile_pool(name="cpool", bufs=6) as cpool, \
                     tc.tile_pool(name="ctmp", bufs=3) as ctmp, \
                     tc.tile_pool(name="dout", bufs=4) as dout, \
                     tc.tile_pool(name="pso_p", bufs=1, space="PSUM") as pso_p, \
                     tc.tile_pool(name="pss_p", bufs=2, space="PSUM") as pss_p, \
                     tc.tile_pool(name="psq_p", bufs=1, space="PSUM") as psq_p, \
                     tc.tile_pool(name="psop_p", bufs=2, space="PSUM") as psop_p:
                    for j in range(NJ):
                        sl = slice(512 * j, 512 * (j + 1))
                        nkb = 4 * j + 4
                        npair = nkb // 2
                        for h in range(NH):
                            pso = pso_p.tile([D, 512], f32, tag="o")
                            # kb tiles processed in pairs: both score matmuls
                            # write one 2-bank psum tile, one Exp covers
                            # [128, 1024], diagonal masks applied pairwise.
                            # Emit pair t+1's scores before pair t's AV so PE
                            # never waits on ACT's exp.
                            se_l = {}

                            def score_pair(t):
                                pss = pss_p.tile([D, 2, 512], f32, tag="s")
                                for i in (0, 1):
                                    kb = 2 * t + i
                                    nc.tensor.matmul(
                                        pss[:, i, :],
                                        kT[:, 128 * kb:128 * (kb + 1)],
                                        qT[:, h, sl], start=True, stop=True)
                                se = cpool.tile([D, 2, 512], f32r, tag="se")
                                nc.scalar.activation(se, pss, Act.Exp,
                                                     scale=float(inv_sqrt_d))
                                if t == 2 * j:
                                    nc.vector.tensor_mul(
                                        se, se, mask_t[:, 0:2, :])
                                elif t == 2 * j + 1:
                                    nc.vector.tensor_mul(
                                        se, se, mask_t[:, 2:4, :])
                                se_l[t] = se

                            score_pair(0)
                            for t in range(npair):
                                if t + 1 < npair:
                                    score_pair(t + 1)
                                se = se_l.pop(t)
                                for i in (0, 1):
                                    kb = 2 * t + i
                                    nc.tensor.matmul(pso, veff[:, kb, :],
                                                     se[:, i, :],
                                                     start=(kb == 0),
                                                     stop=(kb == nkb - 1))

                            # copy u out early so the psum slot frees without
                            # waiting on the normalization chain.
                            u_f = cpool.tile([D, 512], f32, tag="uf")
                            nc.scalar.copy(u_f, pso)
                            # The softmax 1/den and the RMS rsqrt fold into a
                            # single scale: final = u * rsqrt(ssq/128 + eps*den^2)
                            # * rmsw * 0.8.  eps*den^2 <= ~6e-4 of ssq/128, far
                            # below fp32r matmul noise, so den is not computed
                            # at all: comb = exp(-0.5*ln(ssq/128)).
                            sq = cpool.tile([D, 512], f32r, tag="sq")
                            nc.vector.tensor_mul(sq, u_f, u_f)
                            psss = psq_p.tile([D, 512], f32, tag="ss")
                            nc.tensor.matmul(psss, ones_t, sq,
                                             start=True, stop=True)
                            lnr = ctmp.tile([D, 512], f32, tag="lnr")
                            nc.scalar.activation(lnr, psss, Act.Ln,
                                                 scale=1.0 / 128.0)
                            comb = ctmp.tile([D, 512], f32, tag="comb")
                            nc.scalar.activation(comb, lnr, Act.Exp, scale=-0.5)
                            ft = ctmp.tile([D, 512], f32, tag="ft")
                            nc.vector.tensor_mul(ft, u_f, comb)
                            nc.vector.tensor_scalar_mul(
                                finalT[:, h, sl], ft, rmsw_t)

                        # o_proj for the four l-blocks of this q-slice (all
                        # heads of finalT[:, :, sl] are now written); keeps PE
                        # fed while the next slice's attention ACT/DVE runs.
                        for lb in range(4 * j, 4 * j + 4):
                            lsl = slice(128 * lb, 128 * (lb + 1))
                            for osl in range(HID // 512):
                                osl_s = slice(512 * osl, 512 * (osl + 1))
                                ps = psop_p.tile([D, 512], f32, tag="op")
                                for h in range(NH):
                                    nc.tensor.matmul(ps, finalT[:, h, lsl],
                                                     wo_t[:, h, osl_s],
                                                     start=(h == 0),
                                                     stop=(h == NH - 1))
                                ob = dout.tile([D, 512], f32, tag="ob")
                                nc.scalar.copy(ob, ps)
                                nc.sync.dma_start(out=part[lsl, osl_s], in_=ob)

    nc.finalize()
    return nc


def _causal_masks():
    m = np.zeros((4, D, 512), np.float32)
    for r in range(4):
        for p in range(D):
            q0 = 128 * r + p
            if q0 < 512:
                m[r, p, q0:] = 1.0
    return m


def kernel(hidden_states, cos, sin, Wq, Wk, Wv, Wo,
           lambda_q1, lambda_k1, lambda_q2, lambda_k2, rms_weight):
    from concourse.bass_utils import run_bass_kernel_spmd

    length = hidden_states.shape[1]
    if length not in _CACHE:
        _CACHE[length] = _build(length)
    nc = _CACHE[length]

    hidden_states = np.asarray(hidden_states, np.float32)
    cos = np.asarray(cos, np.float32)
    sin = np.asarray(sin, np.float32)

    lam_full = np.float32(
        np.exp(np.float32(np.dot(np.asarray(lambda_q1, np.float32),
                                 np.asarray(lambda_k1, np.float32)))
               + np.float32(np.dot(np.asarray(lambda_q2, np.float32),
                                   np.asarray(lambda_k2, np.float32))))
        + np.float32(LAMBDA_INIT))
    lam_arr = np.full((D, 1), lam_full, np.float32)
    rmsw_arr = np.ascontiguousarray(
        (np.asarray(rms_weight, np.float32)
         * np.float32(1.0 - LAMBDA_INIT)).reshape(D, 1))
    masks = _causal_masks()

    Wq = np.asarray(Wq, np.float32)
    Wk = np.asarray(Wk, np.float32)
    Wv = np.asarray(Wv, np.float32)
    Wo = np.asarray(Wo, np.float32)

    in_maps = []
    for b in range(B):
        hsT_b = np.ascontiguousarray(hidden_states[b].T)
        cosT_b = np.ascontiguousarray(cos[b].T)
        sinT_b = np.ascontiguousarray(sin[b].T)
        for t in range(4):
            vlo, vhi = t // 2, t // 2 + 2
            in_maps.append({
                "hsT": hsT_b,
                "cosT": cosT_b,
                "sinT": sinT_b,
                "wq": np.ascontiguousarray(Wq[:, 512 * t:512 * (t + 1)]),
                "wk": np.ascontiguousarray(Wk[:, 128 * t:128 * (t + 1)]),
                "wv": np.ascontiguousarray(np.concatenate(
                    [Wv[:, 128 * vlo:128 * (vlo + 1)],
                     Wv[:, 128 * vhi:128 * (vhi + 1)]], axis=1)),
                "wo": np.ascontiguousarray(Wo[512 * t:512 * (t + 1), :]),
                "lam": lam_arr,
                "rmsw": rmsw_arr,
                "masks": masks,
            })

    trace = bool(os.environ.get("DIFFATTN_TRACE"))
    res = run_bass_kernel_spmd(nc, in_maps, list(range(8)), trace=trace)
    kernel.last_results = res

    out = np.empty((B, length, HID), np.float32)
    for b in range(B):
        acc = res.results[4 * b]["part"].astype(np.float32).copy()
        for t in range(1, 4):
            acc += res.results[4 * b + t]["part"]
        out[b] = acc
    return out


# revision 34
# speedup vs baseline: 1.3748x; 1.0322x over previous
"""Differential-Transformer attention (DiffAttn) Trainium2 Bass kernel.

Sharding: 8 cores = 2 (batch) x 4 (head-group tensor parallel).
Core c = 4*b + t handles batch b, query heads 4t..4t+3, kv head t,
and the two v-heads its query heads need (t//2 and t//2+2).
o_proj is row-parallel: each core returns a partial [L, HID] product;
the host sums the 4 partials per batch (the "unshard" step).

All matmuls run as float32r (fp32 data, reduced-precision multiply,
full PE speed at moving-dim >= 256). Softmax is computed without
max-subtraction (score magnitudes are bounded ~5, exp is safe in fp32)
on transposed score tiles S^T[k, q] so that the AV matmul needs no
transposes. The softmax denominator and the RMS-norm sum-of-squares
are computed with all-ones stationary matmuls which replicate the
result across all 128 partitions, so the combined normalization scale
  comb = rsqrt(ssq + 128*eps*den^2)       (algebraically exact fold of
                                           1/den, RMS rsqrt, and eps)
is computed on full [128, 512] tiles; rsqrt is exp(-0.5*ln(r)) so the
scalar engine only ever uses one LUT table set (exp/ln/copy/square).
"""

import os
import sys

import numpy as np

for _p in ("/opt/trn_rl_repo",):
    if _p not in sys.path and os.path.isdir(_p):
        sys.path.insert(0, _p)

B = 2
L = 2048
HID = 2048
D = 128
H = 16
NH = 4            # query heads per core
CT = HID // 128   # contraction tiles for the projections
EPS = 1e-6
LAMBDA_INIT = 0.2

_CACHE = {}


def _build(length=L):
    from concourse import bacc
    import concourse.mybir as mybir
    import concourse.tile as tile

    f32 = mybir.dt.float32
    f32r = mybir.dt.float32r
    Act = mybir.ActivationFunctionType

    # All ACT functions used here (Exp, Ln, Square, Copy) live together in
    # the 'natural_log_exp_and_others' LUT set, but the table-load pass
    # assigns each activation the first set containing its function, which
    # alternates sets and inserts a ~1.3us table reload per switch (~42us).
    # Restrict the pass's view to that one set (keeping list positions so
    # act_func_set_id indices stay valid) -> exactly one load total.
    _orig_tables = bacc.get_activation_tables

    def _only_ln_exp(arch):
        t = _orig_tables(arch)
        keep = "natural_log_exp_and_others"
        if keep not in t:
            return t
        return {name: (s if name == keep else set()) for name, s in t.items()}

    bacc.get_activation_tables = _only_ln_exp
    try:
        return _build_inner(length, bacc, mybir, tile, f32, f32r, Act)
    finally:
        bacc.get_activation_tables = _orig_tables


def _build_inner(length, bacc, mybir, tile, f32, f32r, Act):
    NJ = length // 512    # q-slices
    NLB = length // 128   # l/k blocks

    nc = bacc.Bacc()
    hsT = nc.dram_tensor("hsT", [HID, length], f32, kind="ExternalInput")
    cosT = nc.dram_tensor("cosT", [D, length], f32, kind="ExternalInput")
    sinT = nc.dram_tensor("sinT", [D, length], f32, kind="ExternalInput")
    wq = nc.dram_tensor("wq", [HID, NH * D], f32, kind="ExternalInput")
    wk = nc.dram_tensor("wk", [HID, D], f32, kind="ExternalInput")
    wv = nc.dram_tensor("wv", [HID, 2 * D], f32, kind="ExternalInput")
    wo = nc.dram_tensor("wo", [NH * D, HID], f32, kind="ExternalInput")
    lam = nc.dram_tensor("lam", [D, 1], f32, kind="ExternalInput")
    rmsw = nc.dram_tensor("rmsw", [D, 1], f32, kind="ExternalInput")
    masks = nc.dram_tensor("masks", [4, D, 512], f32, kind="ExternalInput")
    part = nc.dram_tensor("part", [length, HID], f32, kind="ExternalOutput")

    inv_sqrt_d = 1.0 / np.sqrt(np.float32(D))

    with tile.TileContext(nc) as tc:
        with tc.tile_pool(name="persist", bufs=1) as persist:
            qT = persist.tile([D, NH, length], f32r, tag="qT")
            kT = persist.tile([D, length], f32r, tag="kT")
            veff = persist.tile([D, NLB, D], f32r, tag="veff")
            mask_t = persist.tile([D, 4, 512], f32r, tag="mask")
            lam_t = persist.tile([D, 1], f32, tag="lam")
            rmsw_t = persist.tile([D, 1], f32, tag="rmsw")
            ones_t = persist.tile([D, D], f32r, tag="ones")
            ones_stage = persist.tile([D, D], f32, tag="ones_stage")

            nc.sync.dma_start(out=lam_t, in_=lam[:, :])

            # ---------------- Phase B: projections + RoPE -----------------
            with tc.tile_pool(name="wpool", bufs=1) as wpool, \
                 tc.tile_pool(name="hspool", bufs=26) as hspool, \
                 tc.tile_pool(name="stpool", bufs=10) as stpool, \
                 tc.tile_pool(name="cspool", bufs=2) as cspool, \
                 tc.tile_pool(name="btmp", bufs=3) as btmp, \
                 tc.tile_pool(name="bpsum", bufs=3, space="PSUM") as bpsum, \
                 tc.tile_pool(name="vpsum", bufs=2, space="PSUM") as vpsum:

                wq_t = wpool.tile([D, CT, NH * D], f32r, tag="wq")
                wk_t = wpool.tile([D, CT, D], f32r, tag="wk")
                wv_t = wpool.tile([D, CT, 2 * D], f32r, tag="wv")
                wq_r = wq.rearrange("(c p) m -> p c m", p=D)
                wk_r = wk.rearrange("(c p) m -> p c m", p=D)
                wv_r = wv.rearrange("(c p) m -> p c m", p=D)

                def load_hs_chunk(c, j):
                    st = stpool.tile([D, 512], f32, tag="stage")
                    ch = hspool.tile([D, 512], f32r, tag="hs")
                    nc.sync.dma_start(
                        out=st, in_=hsT[128 * c:128 * (c + 1),
                                        512 * j:512 * (j + 1)])
                    nc.scalar.copy(out=ch, in_=st)
                    return ch

                # interleave weight-chunk and first-slice hs-chunk loads so
                # the first projection matmul starts as soon as c=0 lands;
                # wv is deferred out of the critical startup window.  All
                # f32->f32r rounding copies run on ACT (idle in this phase).
                chunks0 = []
                for c in range(CT):
                    st = stpool.tile([D, 512], f32, tag="stage")
                    nc.sync.dma_start(out=st[:, 0:NH * D], in_=wq_r[:, c, :])
                    nc.scalar.copy(out=wq_t[:, c, :], in_=st[:, 0:NH * D])
                    st = stpool.tile([D, 512], f32, tag="stage")
                    nc.sync.dma_start(out=st[:, 0:D], in_=wk_r[:, c, :])
                    nc.scalar.copy(out=wk_t[:, c, :], in_=st[:, 0:D])
                    chunks0.append(load_hs_chunk(c, 0))
                for c in range(CT):
                    st = stpool.tile([D, 512], f32, tag="stage")
                    nc.sync.dma_start(out=st[:, 0:2 * D], in_=wv_r[:, c, :])
                    nc.scalar.copy(out=wv_t[:, c, :], in_=st[:, 0:2 * D])

                for j in range(NJ):
                    sl = slice(512 * j, 512 * (j + 1))
                    chunks = chunks0 if j == 0 else [
                        load_hs_chunk(c, j) for c in range(CT)]
                    cos_s = cspool.tile([D, 512], f32, tag="cos")
                    sin_s = cspool.tile([D, 512], f32, tag="sin")
                    nc.sync.dma_start(out=cos_s, in_=cosT[:, sl])
                    nc.sync.dma_start(out=sin_s, in_=sinT[:, sl])

                    # q heads + k head in [D, l] (transposed) layout
                    for db in range(NH + 1):
                        ps = bpsum.tile([D, 512], f32, tag="qk")
                        for c in range(CT):
                            lhsT = (wq_t[:, c, 128 * db:128 * (db + 1)]
                                    if db < NH else wk_t[:, c, :])
                            nc.tensor.matmul(ps, lhsT, chunks[c],
                                             start=(c == 0), stop=(c == CT - 1))
                        # RoPE: rotate_half swaps partition halves; sin's two
                        # halves are identical (emb = concat([freqs, freqs])),
                        # so rot(q)*sin lands via partition-swapped reads of
                        # the PSUM tile (PSUM+SBUF input pairs are exempt from
                        # the same-base-partition rule).
                        t1 = btmp.tile([D, 512], f32, tag="t1")
                        t2 = btmp.tile([D, 512], f32, tag="t2")
                        nc.vector.tensor_mul(t1, ps, cos_s)
                        nc.vector.tensor_mul(t2[0:64, :], ps[64:128, :],
                                             sin_s[0:64, :])
                        nc.vector.tensor_mul(t2[64:128, :], ps[0:64, :],
                                             sin_s[64:128, :])
                        if db < NH:
                            dst_lo = qT[0:64, db, sl]
                            dst_hi = qT[64:128, db, sl]
                        else:
                            dst_lo = kT[0:64, sl]
                            dst_hi = kT[64:128, sl]
                        nc.vector.tensor_sub(dst_lo, t1[0:64, :], t2[0:64, :])
                        nc.vector.tensor_add(dst_hi, t1[64:128, :],
                                             t2[64:128, :])

                    # v in natural [l, d] layout; combine v_lo - lam*v_hi
                    for i in range(4):
                        kb = 4 * j + i
                        psv = vpsum.tile([D, 2 * D], f32, tag="v")
                        for c in range(CT):
                            nc.tensor.matmul(
                                psv, chunks[c][:, 128 * i:128 * (i + 1)],
                                wv_t[:, c, :],
                                start=(c == 0), stop=(c == CT - 1))
                        tv = btmp.tile([D, D], f32, tag="tv")
                        nc.vector.tensor_scalar_mul(tv, psv[:, D:2 * D], lam_t)
                        nc.vector.tensor_sub(veff[:, kb, :], psv[:, 0:D], tv)

            # ---------------- attention + o_proj --------------------------
            with tc.tile_pool(name="cdpool", bufs=1) as cdpool, \
                 tc.tile_pool(name="wostage", bufs=4) as wostage:
                finalT = cdpool.tile([D, NH, length], f32r, tag="finalT")
                wo_t = cdpool.tile([D, NH, HID], f32r, tag="wo")

                nc.sync.dma_start(out=mask_t.bitcast(f32),
                                  in_=masks.rearrange("m p q -> p m q"))
                nc.sync.dma_start(out=rmsw_t, in_=rmsw[:, :])
                nc.vector.memset(ones_stage, 1.0)
                nc.vector.tensor_copy(out=ones_t, in_=ones_stage)

                # prefetch + round wo during the attention phase
                wo_r = wo.rearrange("(h p) m -> p h m", p=D)
                for h in range(NH):
                    for q4 in range(HID // 512):
                        st = wostage.tile([D, 512], f32, tag="wost")
                        nc.sync.dma_start(
                            out=st, in_=wo_r[:, h, 512 * q4:512 * (q4 + 1)])
                        nc.vector.tensor_copy(
                            out=wo_t[:, h, 512 * q4:512 * (q4 + 1)], in_=st)

                with tc.tile_pool(name="cpool", bufs=6) as cpool, \
                     tc.tile_pool(name="ctmp", bufs=3) as ctmp, \
                     tc.tile_pool(name="dout", bufs=4) as dout, \
                     tc.tile_pool(name="pso_p", bufs=1, space="PSUM") as pso_p, \
                     tc.tile_pool(name="pss_p", bufs=2, space="PSUM") as pss_p, \
                     tc.tile_pool(name="psq_p", bufs=1, space="PSUM") as psq_p, \
                     tc.tile_pool(name="psop_p", bufs=2, space="PSUM") as psop_p:
                    for j in reversed(range(NJ)):
                        sl = slice(512 * j, 512 * (j + 1))
                        nkb = 4 * j + 4
                        npair = nkb // 2
                        for h in range(NH):
                            pso = pso_p.tile([D, 512], f32, tag="o")
                            # kb tiles processed in pairs: both score matmuls
                            # write one 2-bank psum tile, one Exp covers
                            # [128, 1024], diagonal masks applied pairwise.
                            # Emit pair t+1's scores before pair t's AV so PE
                            # never waits on ACT's exp.
                            se_l = {}

                            def score_pair(t):
                                pss = pss_p.tile([D, 2, 512], f32, tag="s")
                                for i in (0, 1):
                                    kb = 2 * t + i
                                    nc.tensor.matmul(
                                        pss[:, i, :],
                                        kT[:, 128 * kb:128 * (kb + 1)],
                                        qT[:, h, sl], start=True, stop=True)
                                se = cpool.tile([D, 2, 512], f32r, tag="se")
                                nc.scalar.activation(se, pss, Act.Exp,
                                                     scale=float(inv_sqrt_d))
                                if t == 2 * j:
                                    nc.vector.tensor_mul(
                                        se[:, 0, 0:128], se[:, 0, 0:128],
                                        mask_t[:, 0, 0:128])
                                    nc.vector.tensor_mul(
                                        se[:, 1, 0:256], se[:, 1, 0:256],
                                        mask_t[:, 1, 0:256])
                                elif t == 2 * j + 1:
                                    nc.vector.tensor_mul(
                                        se[:, 0, 0:384], se[:, 0, 0:384],
                                        mask_t[:, 2, 0:384])
                                    nc.vector.tensor_mul(
                                        se[:, 1, :], se[:, 1, :],
                                        mask_t[:, 3, :])
                                se_l[t] = se

                            score_pair(0)
                            for t in range(npair):
                                if t + 1 < npair:
                                    score_pair(t + 1)
                                se = se_l.pop(t)
                                for i in (0, 1):
                                    kb = 2 * t + i
                                    nc.tensor.matmul(pso, veff[:, kb, :],
                                                     se[:, i, :],
                                                     start=(kb == 0),
                                                     stop=(kb == nkb - 1))

                            # copy u out early so the psum slot frees without
                            # waiting on the normalization chain.
                            u_f = cpool.tile([D, 512], f32, tag="uf")
                            nc.vector.tensor_copy(u_f, pso)
                            # The softmax 1/den and the RMS rsqrt fold into a
                            # single scale: final = u * rsqrt(ssq/128 +
                            # eps*den^2) * rmsw * 0.8.  eps*den^2 <= ~6e-4 of
                            # ssq/128, far below fp32r matmul noise, so den is
                            # not computed at all: comb = exp(-0.5*ln(ssq/128)).
                            sq = cpool.tile([D, 512], f32r, tag="sq")
                            nc.vector.tensor_mul(sq, u_f, u_f)
                            psss = psq_p.tile([D, 512], f32, tag="ss")
                            nc.tensor.matmul(psss, ones_t, sq,
                                             start=True, stop=True)
                            lnr = ctmp.tile([D, 512], f32, tag="lnr")
                            nc.scalar.activation(lnr, psss, Act.Ln,
                                                 scale=1.0 / 128.0)
                            comb = ctmp.tile([D, 512], f32, tag="comb")
                            nc.scalar.activation(comb, lnr, Act.Exp, scale=-0.5)
                            ft = ctmp.tile([D, 512], f32, tag="ft")
                            nc.vector.tensor_mul(ft, u_f, comb)
                            nc.vector.tensor_scalar_mul(
                                finalT[:, h, sl], ft, rmsw_t)

                        # o_proj for the four l-blocks of this q-slice (all
                        # heads of finalT[:, :, sl] are now written); keeps PE
                        # fed while the next slice's attention ACT/DVE runs.
                        for lb in range(4 * j, 4 * j + 4):
                            lsl = slice(128 * lb, 128 * (lb + 1))
                            for osl in range(HID // 512):
                                osl_s = slice(512 * osl, 512 * (osl + 1))
                                ps = psop_p.tile([D, 512], f32, tag="op")
                                for h in range(NH):
                                    nc.tensor.matmul(ps, finalT[:, h, lsl],
                                                     wo_t[:, h, osl_s],
                                                     start=(h == 0),
                                                     stop=(h == NH - 1))
                                ob = dout.tile([D, 512], f32, tag="ob")
                                nc.vector.tensor_copy(ob, ps)
                                nc.sync.dma_start(out=part[lsl, osl_s], in_=ob)

    nc.finalize()
    return nc


def _causal_masks():
    m = np.zeros((4, D, 512), np.float32)
    for r in range(4):
        for p in range(D):
            q0 = 128 * r + p
            if q0 < 512:
                m[r, p, q0:] = 1.0
    return m


def kernel(hidden_states, cos, sin, Wq, Wk, Wv, Wo,
           lambda_q1, lambda_k1, lambda_q2, lambda_k2, rms_weight):
    from concourse.bass_utils import run_bass_kernel_spmd

    length = hidden_states.shape[1]
    if length not in _CACHE:
        _CACHE[length] = _build(length)
    nc = _CACHE[length]

    hidden_states = np.asarray(hidden_states, np.float32)
    cos = np.asarray(cos, np.float32)
    sin = np.asarray(sin, np.float32)

    lam_full = np.float32(
        np.exp(np.float32(np.dot(np.asarray(lambda_q1, np.float32),
                                 np.asarray(lambda_k1, np.float32)))
               + np.float32(np.dot(np.asarray(lambda_q2, np.float32),
                                   np.asarray(lambda_k2, np.float32))))
        + np.float32(LAMBDA_INIT))
    lam_arr = np.full((D, 1), lam_full, np.float32)
    rmsw_arr = np.ascontiguousarray(
        (np.asarray(rms_weight, np.float32)
         * np.float32(1.0 - LAMBDA_INIT)).reshape(D, 1))
    masks = _causal_masks()

    Wq = np.asarray(Wq, np.float32)
    Wk = np.asarray(Wk, np.float32)
    Wv = np.asarray(Wv, np.float32)
    Wo = np.asarray(Wo, np.float32)

    in_maps = []
    for b in range(B):
        hsT_b = np.ascontiguousarray(hidden_states[b].T)
        cosT_b = np.ascontiguousarray(cos[b].T)
        sinT_b = np.ascontiguousarray(sin[b].T)
        for t in range(4):
            vlo, vhi = t // 2, t // 2 + 2
            in_maps.append({
                "hsT": hsT_b,
                "cosT": cosT_b,
                "sinT": sinT_b,
                "wq": np.ascontiguousarray(Wq[:, 512 * t:512 * (t + 1)]),
                "wk": np.ascontiguousarray(Wk[:, 128 * t:128 * (t + 1)]),
                "wv": np.ascontiguousarray(np.concatenate(
                    [Wv[:, 128 * vlo:128 * (vlo + 1)],
                     Wv[:, 128 * vhi:128 * (vhi + 1)]], axis=1)),
                "wo": np.ascontiguousarray(Wo[512 * t:512 * (t + 1), :]),
                "lam": lam_arr,
                "rmsw": rmsw_arr,
                "masks": masks,
            })

    trace = bool(os.environ.get("DIFFATTN_TRACE"))
    res = run_bass_kernel_spmd(nc, in_maps, list(range(8)), trace=trace)
    kernel.last_results = res

    out = np.empty((B, length, HID), np.float32)
    for b in range(B):
        acc = res.results[4 * b]["part"].astype(np.float32).copy()
        for t in range(1, 4):
            acc += res.results[4 * b + t]["part"]
        out[b] = acc
    return out
